# revision 30
# baseline (speedup 1.0000x reference)
"""Trainium2 Bass kernel for nn_DVGOMoE (moe_routing).

Strategy
--------
Data-parallel over rays: the 8192 rays (1048576 points, sorted by ray_id) are
split into 8 contiguous ray ranges with ~equal point counts; each NeuronCore
processes its range independently (rays never span cores). Gate + expert MLP
weights are replicated on every core.

Per core, two passes:

Pass 1 (point-major, tiled by 2048-point superchunks):
  * x^T is staged host-side as [6, pts] feature-major chunks at 4 partition
    groups (rows 32i..32i+5) so 4 row-tiled K=6 matmuls run concurrently on
    the PE array.
  * gate logits via stationary-x^T matmuls -> [128 pts, 8] point-major; top-2
    + renormalization computed with reduce_max / is_equal one-hots and
    sigma(z1-z2) = exp(-ln(1+exp(-(z1-z2)))).
  * per expert e (8 experts + density "expert" 8): h_e^T = relu(We1_e^T x + b)
    via row-tiled matmuls -> PSUM, evacuated (fused bias+relu) to SBUF split
    across DVE and ACT; second matmul with stationary h_e^T block and
    streamed [Walpha_e|Wrgb_e] producing point-major [128, 4] outputs.
  * all nonlinearities use only Exp/Ln (one ACT table set):
      sigmoid(z) = exp(-ln(1+exp(-z)))
      raw2alpha(y) = 1 - (1+exp(y+ACT_SHIFT))^(-1/2)    [INTERVAL = 0.5]
                   = 1 - exp(-0.5*ln(1+exp(y+ACT_SHIFT)))
  * gate-mixed quantities (sum_e g_e * q_e, sum_e g_e * rgb_e) and the
    density alpha0 are staged to DRAM per point (5 floats/point).

Pass 2 (ray-major): rays are padded to S=192 slots. For each tile of 128
rays, the 5 staged quantities are gathered with indirect DMA (row r =
staging[start_r : start_r+192], tail masked). Transmittance is an inclusive
per-ray cumsum of ln(1-alpha) via tensor_tensor_scan; weights, the keep mask
(w0 > 1e-4, folded multiplicatively), weighted rgb sums (free-dim reduce) and
the background composite all stay per-partition. Output is [128, 3] per tile.
"""

import numpy as np

import concourse.bass as bass
import concourse.bacc as bacc
import concourse.tile as tile
from concourse import mybir
from concourse.bass_utils import run_bass_kernel_spmd
from concourse.tile_rust import add_dep_helper

F32 = mybir.dt.float32
I32 = mybir.dt.int32
AF = mybir.ActivationFunctionType
OP = mybir.AluOpType
AX = mybir.AxisListType

# problem constants
P_TOT = 1048576
N_RAYS = 8192
E = 8
H = 128
ACT_SHIFT = -4.0
THRESH = 1e-4
NCORE = 8

# kernel layout constants
SC = 2048                 # points per superchunk
NSC = 65                  # superchunks per core
P_PAD = NSC * SC          # padded points per core (133120)
NRT = 9                   # ray tiles per core
R_PAD = NRT * 128         # padded rays per core (1152)
S = 192                   # slots per ray (max ray len 174 in this data)
STG = P_PAD + 256         # staging length (gather overrun margin)

_H_DT = F32               # fp32 h keeps kernel error at the reference noise floor


def build_nc(nsc=NSC, nrt=NRT, h_dt=_H_DT, num_devices=NCORE):
    nc = bacc.Bacc("TRN2", target_bir_lowering=False, debug=False,
                   num_devices=num_devices)

    # ---- DRAM I/O ----
    xt = nc.dram_tensor("xt", [nsc, 24, 512], F32, kind="ExternalInput")
    w1 = nc.dram_tensor("w1", [128, 9, 128], F32, kind="ExternalInput")
    wg = nc.dram_tensor("wg", [128, 8], F32, kind="ExternalInput")
    w2 = nc.dram_tensor("w2", [128, 36], F32, kind="ExternalInput")
    be1 = nc.dram_tensor("be1", [128, 9], F32, kind="ExternalInput")
    bgc = nc.dram_tensor("bgc", [128, 3], F32, kind="ExternalInput")
    starts = nc.dram_tensor("starts", [nrt, 128], I32, kind="ExternalInput")
    pmask = nc.dram_tensor("pmask", [nrt, 128, S], F32, kind="ExternalInput")
    outrgb = nc.dram_tensor("outrgb", [nrt, 128, 3], F32, kind="ExternalOutput")

    stg_len = nsc * SC + 256
    stg_names = ["sa0", "sq", "sr", "sg2", "sb2"]
    stg = [nc.dram_tensor(n, [stg_len], F32, kind="Internal")
           for n in stg_names]

    stg_sem_cm = nc.semaphore()
    stg_sem = stg_sem_cm.__enter__()

    with tile.TileContext(nc) as tc:
        consts = tc.alloc_tile_pool(name="consts", bufs=1)
        sb = tc.alloc_tile_pool(name="sb", bufs=3)
        xap = tc.alloc_tile_pool(name="xap", bufs=4)
        hpool = tc.alloc_tile_pool(name="hpool", bufs=3)
        ps = tc.alloc_tile_pool(name="ps", bufs=1, space="PSUM")
        hps = tc.alloc_tile_pool(name="hps", bufs=1, space="PSUM")
        ps2 = tc.alloc_tile_pool(name="ps2", bufs=2, space="PSUM")

        # ---- load constants ----
        w1c = consts.tile([128, 9, 128], F32)
        wgc = consts.tile([128, 8], F32)
        w2c = consts.tile([128, 36], h_dt)
        be1c = consts.tile([128, 9], F32)
        bgcc = consts.tile([128, 3], F32)
        onesS = consts.tile([128, S], F32)
        shiftc = consts.tile([128, 1], F32)
        nc.vector.memset(shiftc[:], ACT_SHIFT)
        nc.sync.dma_start(w1c[:], w1.ap())
        nc.sync.dma_start(wgc[:], wg.ap())
        if h_dt == F32:
            nc.sync.dma_start(w2c[:], w2.ap())
        else:
            nc.gpsimd.dma_start(w2c[:], w2.ap())  # SWDGE casts f32 -> bf16
        nc.sync.dma_start(be1c[:], be1.ap())
        nc.sync.dma_start(bgcc[:], bgc.ap())
        nc.vector.memset(onesS[:], 1.0)

        # zero the staging overrun tail (gathers read past the last point;
        # uninitialized DRAM could be non-finite and NaN*0 = NaN)
        stg_writes = [[] for _ in range(5)]  # explicit RAW deps for gathers
        n_stg_writes = [0]
        ztail = consts.tile([128, 2], F32)
        nc.vector.memset(ztail[:], 0.0)
        for qi in range(5):
            tail = stg[qi].ap()[bass.ds(nsc * SC, 256)].rearrange(
                "(b p) -> p b", p=128)
            w = nc.sync.dma_start(tail, ztail[:])
            stg_writes[qi].append(w.ins)
            n_stg_writes[0] += 1

        xt_r = xt.ap().rearrange("a r f -> (a r) f")

        # ================= PASS 1 =================
        def sc_body(s):
            # x^T superchunk: rows 32i..32i+5 = chunk i features
            xa = xap.tile([128, 512], F32, tag="xa", name="xa")
            for i in range(4):
                nc.sync.dma_start(
                    xa[32 * i:32 * i + 6, :],
                    xt_r[bass.ds(s * 24 + 6 * i, 6), :])

            # ---- gate: stationary-x^T blocks, out [128 pts, 8] ----
            zg_ps = ps2.tile([128, 16, 8], F32, tag="zg", name="zg_ps")
            for i in range(4):
                for b in range(4):
                    k = 4 * i + b
                    nc.tensor.matmul(
                        zg_ps[:, k, :],
                        lhsT=xa[32 * i:32 * i + 6, 128 * b:128 * (b + 1)],
                        rhs=wgc[32 * i:32 * i + 6, :],
                        start=True, stop=True, tile_position=(32 * i, 0))
            zg = sb.tile([128, 16, 8], F32, tag="zg_sb", name="zg")
            nc.vector.tensor_copy(zg[:], zg_ps[:])

            # top-2 one-hots + weights
            m1 = sb.tile([128, 16], F32, tag="m1", name="m1")
            nc.vector.tensor_reduce(out=m1[:], in_=zg[:], axis=AX.X, op=OP.max)
            m1b = bass.AP(m1.tensor, m1[:].offset,
                          [m1[:].ap[0], m1[:].ap[1], [0, 8]])
            eq1 = sb.tile([128, 16, 8], F32, tag="eq1", name="eq1")
            nc.vector.tensor_tensor(out=eq1[:], in0=zg[:], in1=m1b,
                                    op=OP.is_equal)
            zm = sb.tile([128, 16, 8], F32, tag="zm", name="zm")
            nc.vector.scalar_tensor_tensor(out=zm[:], in0=eq1[:], scalar=-1e9,
                                           in1=zg[:], op0=OP.mult, op1=OP.add)
            m2 = sb.tile([128, 16], F32, tag="m2", name="m2")
            nc.vector.tensor_reduce(out=m2[:], in_=zm[:], axis=AX.X, op=OP.max)
            m2b = bass.AP(m2.tensor, m2[:].offset,
                          [m2[:].ap[0], m2[:].ap[1], [0, 8]])
            eq2 = sb.tile([128, 16, 8], F32, tag="eq2", name="eq2")
            nc.vector.tensor_tensor(out=eq2[:], in0=zg[:], in1=m2b,
                                    op=OP.is_equal)
            d12 = sb.tile([128, 16], F32, tag="d12", name="d12")
            nc.vector.tensor_tensor(out=d12[:], in0=m1[:], in1=m2[:],
                                    op=OP.subtract)
            # g1 = sigmoid(d12) via exp/ln
            ge = sb.tile([128, 16], F32, tag="ge", name="ge")
            nc.scalar.activation(ge[:], d12[:], AF.Exp, bias=0.0, scale=-1.0)
            gl = sb.tile([128, 16], F32, tag="gl", name="gl")
            nc.scalar.activation(gl[:], ge[:], AF.Ln, bias=1.0, scale=1.0)
            g1 = sb.tile([128, 16], F32, tag="g1", name="g1")
            nc.scalar.activation(g1[:], gl[:], AF.Exp, bias=0.0, scale=-1.0)
            g1b = bass.AP(g1.tensor, g1[:].offset,
                          [g1[:].ap[0], g1[:].ap[1], [0, 8]])
            # g = eq2 + (eq1 - eq2) * g1
            dq = sb.tile([128, 16, 8], F32, tag="dq", name="dq")
            nc.vector.tensor_tensor(out=dq[:], in0=eq1[:], in1=eq2[:],
                                    op=OP.subtract)
            p1t = sb.tile([128, 16, 8], F32, tag="p1t", name="p1t")
            nc.vector.tensor_tensor(out=p1t[:], in0=dq[:], in1=g1b, op=OP.mult)
            gat = sb.tile([128, 16, 8], F32, tag="gat", name="gat")
            nc.vector.tensor_tensor(out=gat[:], in0=p1t[:], in1=eq2[:],
                                    op=OP.add)

            # ---- experts ----
            o_psA = ps.tile([128, 8, 36], F32, tag="oA", name="o_psA")
            o_psB = ps.tile([128, 8, 36], F32, tag="oB", name="o_psB")
            for e in range(9):
                # bf16 h: [128,512]bf16 = half bank, so 4 chunks fit in 2
                # banks and hbufs=2 double-buffers experts in 4 banks total
                hp = [hps.tile([128, 512], F32, tag=f"hp{i}", name=f"hp{i}")
                      for i in range(4)]
                for i in range(4):
                    nc.tensor.matmul(
                        hp[i][:], lhsT=w1c[32 * i:32 * i + 6, e, :],
                        rhs=xa[32 * i:32 * i + 6, :],
                        start=True, stop=True, tile_position=(32 * i, 0))
                hsb = hpool.tile([128, 2048], h_dt, tag="hsb", name="hsb")
                for i in range(4):
                    dst = hsb[:, 512 * i:512 * (i + 1)]
                    if i % 2 == 0:
                        nc.vector.tensor_scalar(
                            out=dst, in0=hp[i][:], scalar1=be1c[:, e:e + 1],
                            scalar2=0.0, op0=OP.add, op1=OP.max)
                    else:
                        nc.scalar.activation(dst, hp[i][:], AF.Relu,
                                             bias=be1c[:, e:e + 1], scale=1.0)
                for k in range(16):
                    o_ps = o_psA if k < 8 else o_psB
                    nc.tensor.matmul(
                        o_ps[:, k % 8, 4 * e:4 * e + 4],
                        lhsT=hsb[:, 128 * k:128 * (k + 1)],
                        rhs=w2c[:, 4 * e:4 * e + 4],
                        start=True, stop=True)

            # ---- nonlinearities (Exp/Ln only) ----
            vA = o_psA[:].rearrange("p b (s c) -> p b s c", c=4)
            vB = o_psB[:].rearrange("p b (s c) -> p b s c", c=4)
            ua = sb.tile([128, 16, 9], F32, tag="ua", name="ua")
            nc.scalar.activation(ua[:, 0:8, :], vA[:, :, :, 0], AF.Exp,
                                 bias=shiftc[:, :1], scale=1.0)
            nc.scalar.activation(ua[:, 8:16, :], vB[:, :, :, 0], AF.Exp,
                                 bias=shiftc[:, :1], scale=1.0)
            ur = sb.tile([128, 16, 9, 3], F32, tag="ur", name="ur")
            nc.scalar.activation(ur[:, 0:8, :, :], vA[:, :, :, 1:4], AF.Exp,
                                 bias=0.0, scale=-1.0)
            nc.scalar.activation(ur[:, 8:16, :, :], vB[:, :, :, 1:4], AF.Exp,
                                 bias=0.0, scale=-1.0)
            va = sb.tile([128, 16, 9], F32, tag="va", name="va")
            nc.scalar.activation(va[:], ua[:], AF.Ln, bias=1.0, scale=1.0)
            qa = sb.tile([128, 16, 9], F32, tag="qa", name="qa")
            nc.scalar.activation(qa[:], va[:], AF.Exp, bias=0.0, scale=-0.5)
            vr = sb.tile([128, 16, 9, 3], F32, tag="vr", name="vr")
            nc.scalar.activation(vr[:], ur[:], AF.Ln, bias=1.0, scale=1.0)
            sr = sb.tile([128, 16, 9, 3], F32, tag="sr", name="sr")
            nc.scalar.activation(sr[:], vr[:], AF.Exp, bias=0.0, scale=-1.0)

            # ---- gate mixing ----
            gq = sb.tile([128, 16, 8], F32, tag="gq", name="gq")
            nc.vector.tensor_tensor(out=gq[:], in0=qa[:, :, 0:8], in1=gat[:],
                                    op=OP.mult)
            umq = sb.tile([128, 16], F32, tag="umq", name="umq")
            nc.vector.tensor_reduce(out=umq[:], in_=gq[:], axis=AX.X, op=OP.add)
            ga3 = bass.AP(gat.tensor, gat[:].offset, gat[:].ap + [[0, 3]])
            pr = sb.tile([128, 16, 8, 3], F32, tag="pr", name="pr")
            nc.vector.tensor_tensor(out=pr[:], in0=sr[:, :, 0:8, :], in1=ga3,
                                    op=OP.mult)
            umr = sb.tile([128, 16, 3], F32, tag="umr", name="umr")
            nc.vector.tensor_reduce(out=umr[:],
                                    in_=pr[:].rearrange("p k e c -> p k c e"),
                                    axis=AX.X, op=OP.add)
            # density alpha0, thresholded
            a0 = sb.tile([128, 16], F32, tag="a0", name="a0")
            nc.vector.tensor_scalar(out=a0[:], in0=qa[:, :, 8], scalar1=-1.0,
                                    scalar2=1.0, op0=OP.mult, op1=OP.add)
            th = sb.tile([128, 16], F32, tag="th", name="th")
            nc.vector.tensor_scalar(out=th[:], in0=a0[:], scalar1=THRESH,
                                    scalar2=None, op0=OP.is_gt)
            a0f = sb.tile([128, 16], F32, tag="a0f", name="a0f")
            nc.vector.tensor_tensor(out=a0f[:], in0=a0[:], in1=th[:],
                                    op=OP.mult)

            # ---- stage to DRAM ----
            def stage(qi, src_ap):
                dst = stg[qi].ap()[bass.ds(s * SC, SC)].rearrange(
                    "(b p) -> p b", p=128)
                w = nc.sync.dma_start(dst, src_ap)
                stg_writes[qi].append(w.ins)
                n_stg_writes[0] += 1

            stage(0, a0f[:])
            stage(1, umq[:])
            stage(2, umr[:, :, 0])
            stage(3, umr[:, :, 1])
            stage(4, umr[:, :, 2])

        for s in range(nsc):
            sc_body(s)

        # ================= PASS 2 =================
        # hard fence: every pass-2 instruction (incl. the indirect gathers)
        # must observe all pass-1 staging writes
        tc.strict_bb_all_engine_barrier()
        p2 = tc.alloc_tile_pool(name="p2", bufs=2)
        for t in range(nrt):
            st = p2.tile([128, 1], I32, tag="st", name="st")
            nc.sync.dma_start(st[:], starts.ap()[t, :, None])
            msk = p2.tile([128, S], F32, tag="msk", name="msk")
            nc.sync.dma_start(msk[:], pmask.ap()[t])
            gth = []
            for qi in range(5):
                gt = p2.tile([128, S], F32, tag=f"g{qi}", name=f"gt{qi}")
                # [stg_len, 1] view: indirect coef=1, S elems/idx from out
                src = bass.AP(stg[qi], 0, [[1, stg_len], [1, 1]])
                gi = nc.gpsimd.indirect_dma_start(
                    out=gt[:], out_offset=None, in_=src,
                    in_offset=bass.IndirectOffsetOnAxis(ap=st[:, :1], axis=0))
                # Tile only tracks the most recent DRAM writer for the
                # indirect source; add the full RAW edge set explicitly.
                for w in stg_writes[qi]:
                    add_dep_helper(gi.ins, w, reason="staging RAW")
                gth.append(gt)
            pa0, pq, prr, pgg, pbb = gth

            # density pass: w0 and keep
            a0m = p2.tile([128, S], F32, tag="a0m", name="a0m")
            nc.vector.tensor_tensor(out=a0m[:], in0=pa0[:], in1=msk[:],
                                    op=OP.mult)
            la0 = p2.tile([128, S], F32, tag="la0", name="la0")
            nc.scalar.activation(la0[:], a0m[:], AF.Ln, bias=1.0, scale=-1.0)
            lc0 = p2.tile([128, S], F32, tag="lc0", name="lc0")
            nc.vector.tensor_tensor_scan(lc0[:], data0=onesS[:], data1=la0[:],
                                         initial=0.0, op0=OP.mult, op1=OP.add)
            ex0 = p2.tile([128, S], F32, tag="ex0", name="ex0")
            nc.vector.tensor_tensor(out=ex0[:], in0=lc0[:], in1=la0[:],
                                    op=OP.subtract)
            t0 = p2.tile([128, S], F32, tag="t0", name="t0")
            nc.scalar.activation(t0[:], ex0[:], AF.Exp, bias=0.0, scale=1.0)
            w0 = p2.tile([128, S], F32, tag="w0", name="w0")
            nc.vector.tensor_tensor(out=w0[:], in0=a0m[:], in1=t0[:],
                                    op=OP.mult)
            keep = p2.tile([128, S], F32, tag="keep", name="keep")
            nc.vector.tensor_scalar(out=keep[:], in0=w0[:], scalar1=THRESH,
                                    scalar2=None, op0=OP.is_gt)

            # alpha_mix = keep * (1 - umq), clipped
            am = p2.tile([128, S], F32, tag="am", name="am")
            nc.vector.tensor_scalar(out=am[:], in0=pq[:], scalar1=-1.0,
                                    scalar2=1.0, op0=OP.mult, op1=OP.add)
            amk = p2.tile([128, S], F32, tag="amk", name="amk")
            nc.vector.tensor_tensor(out=amk[:], in0=am[:], in1=keep[:],
                                    op=OP.mult)
            nc.vector.tensor_scalar(out=amk[:], in0=amk[:],
                                    scalar1=1.0 - 1e-6, scalar2=None,
                                    op0=OP.min)

            la2 = p2.tile([128, S], F32, tag="la2", name="la2")
            nc.scalar.activation(la2[:], amk[:], AF.Ln, bias=1.0, scale=-1.0)
            lc2 = p2.tile([128, S], F32, tag="lc2", name="lc2")
            nc.vector.tensor_tensor_scan(lc2[:], data0=onesS[:], data1=la2[:],
                                         initial=0.0, op0=OP.mult, op1=OP.add)
            ex2 = p2.tile([128, S], F32, tag="ex2", name="ex2")
            nc.vector.tensor_tensor(out=ex2[:], in0=lc2[:], in1=la2[:],
                                    op=OP.subtract)
            tt = p2.tile([128, S], F32, tag="tt", name="tt")
            nc.scalar.activation(tt[:], ex2[:], AF.Exp, bias=0.0, scale=1.0)
            ww = p2.tile([128, S], F32, tag="ww", name="ww")
            nc.vector.tensor_tensor(out=ww[:], in0=amk[:], in1=tt[:],
                                    op=OP.mult)

            # weighted rgb sums
            sums = p2.tile([128, 3], F32, tag="sums", name="sums")
            for ci, pc in enumerate((prr, pgg, pbb)):
                wr = p2.tile([128, S], F32, tag="wr", name="wr")
                nc.vector.tensor_tensor(out=wr[:], in0=pc[:], in1=ww[:],
                                        op=OP.mult)
                nc.vector.tensor_reduce(out=sums[:, ci:ci + 1], in_=wr[:],
                                        axis=AX.X, op=OP.add)
            ainv = p2.tile([128, 1], F32, tag="ainv", name="ainv")
            nc.scalar.activation(ainv[:], lc2[:, S - 1:S], AF.Exp,
                                 bias=0.0, scale=1.0)
            outt = p2.tile([128, 3], F32, tag="outt", name="outt")
            nc.vector.scalar_tensor_tensor(out=outt[:], in0=bgcc[:],
                                           scalar=ainv[:, :1], in1=sums[:],
                                           op0=OP.mult, op1=OP.add)
            nc.sync.dma_start(outrgb.ap()[t], outt[:])

        p2.release(); ps2.release(); hps.release(); ps.release()
        hpool.release(); xap.release(); sb.release(); consts.release()

    nc.compile()
    return nc


# ---------------- host-side prep ----------------

def host_prep(inputs, nsc=NSC, nrt=NRT):
    """Build per-core in_maps + metadata from full inputs."""
    ray_pts = np.asarray(inputs["ray_pts"], np.float32)
    viewdirs = np.asarray(inputs["viewdirs"], np.float32)
    bg = np.asarray(inputs["bg"], np.float32)
    Wd1 = np.asarray(inputs["Wd1"], np.float32)
    bd1 = np.asarray(inputs["bd1"], np.float32)
    Wd2 = np.asarray(inputs["Wd2"], np.float32)
    Wg = np.asarray(inputs["Wg"], np.float32)
    We1 = np.asarray(inputs["We1"], np.float32)
    be1 = np.asarray(inputs["be1"], np.float32)
    Wrgb = np.asarray(inputs["Wrgb"], np.float32)
    Walpha = np.asarray(inputs["Walpha"], np.float32)
    ray_id = np.asarray(inputs["ray_id"], np.int32)

    P = ray_pts.shape[0]
    N = viewdirs.shape[0]
    counts = np.bincount(ray_id, minlength=N).astype(np.int64)
    ends_cum = np.cumsum(counts)
    starts_all = ends_cum - counts

    # ray-boundary sharding with ~equal point counts
    tgt = np.arange(1, NCORE) * (P / NCORE)
    cut_rays = np.searchsorted(ends_cum, tgt)  # ray index where core ends
    ray_bounds = np.concatenate([[0], cut_rays + 1, [N]])
    pt_bounds = np.concatenate([[0], ends_cum[ray_bounds[1:-1] - 1], [P]])

    # shared constant blocks
    w1 = np.zeros((128, 9, 128), np.float32)
    we_all = np.concatenate(
        [We1, np.concatenate([Wd1, np.zeros((3, H), np.float32)], 0)[None]], 0)
    for i in range(4):
        w1[32 * i:32 * i + 6] = we_all.transpose(1, 0, 2)  # [6, 9, 128]
    wg = np.zeros((128, 8), np.float32)
    for i in range(4):
        wg[32 * i:32 * i + 6] = Wg
    w2 = np.zeros((128, 36), np.float32)
    for e in range(E):
        w2[:, 4 * e] = Walpha[e, :, 0]
        w2[:, 4 * e + 1:4 * e + 4] = Wrgb[e]
    w2[:, 32] = Wd2[:, 0]
    be1s = np.zeros((128, 9), np.float32)
    be1s[:, :8] = be1.T
    be1s[:, 8] = bd1
    bgc = np.broadcast_to(bg[None, :], (128, 3)).copy()

    in_maps, metas = [], []
    for c in range(NCORE):
        r0, r1 = int(ray_bounds[c]), int(ray_bounds[c + 1])
        p0, p1 = int(pt_bounds[c]), int(pt_bounds[c + 1])
        n_pts, n_rays = p1 - p0, r1 - r0
        assert n_pts <= nsc * SC, (n_pts, nsc * SC)
        assert n_rays <= nrt * 128, (n_rays, nrt * 128)

        xt = np.zeros((nsc * SC, 6), np.float32)
        xt[:n_pts, 0:3] = ray_pts[p0:p1]
        xt[:n_pts, 3:6] = viewdirs[ray_id[p0:p1]]
        # [NSC, 24, 512]: row 6i+j = feature j of chunk i
        xt = xt.reshape(nsc, 4, 512, 6).transpose(0, 1, 3, 2).reshape(
            nsc, 24, 512)

        st = np.zeros(nrt * 128, np.int32)
        st[:n_rays] = (starts_all[r0:r1] - p0).astype(np.int32)
        pm = np.zeros((nrt * 128, S), np.float32)
        lens = counts[r0:r1]
        pm[:n_rays] = (np.arange(S)[None, :] < lens[:, None])

        in_maps.append({
            "xt": xt, "w1": w1, "wg": wg, "w2": w2, "be1": be1s, "bgc": bgc,
            "starts": st.reshape(nrt, 128),
            "pmask": pm.reshape(nrt, 128, S),
        })
        metas.append((r0, r1, n_rays))
    return in_maps, metas


_NC_CACHE = {}


def _get_nc():
    key = (NSC, NRT, _H_DT)
    if key not in _NC_CACHE:
        _NC_CACHE[key] = build_nc()
    return _NC_CACHE[key]


def _ensure_ntff_hook():
    """Provide antenv.axon_hooks (NTFF profiling shim) if the image lacks it."""
    try:
        from antenv import axon_hooks  # noqa: F401
        return
    except ImportError:
        pass
    import sys
    import types
    try:
        from trn_agent_boot.trn_boot import _ntff_profile_via_ctypes
        hook = _ntff_profile_via_ctypes("/opt/axon/libaxon_pjrt.so")
    except Exception:
        hook = None
    mod = types.ModuleType("antenv.axon_hooks")
    mod.get_axon_ntff_profile_hook = lambda: hook
    mod.set_axon_ntff_profile_hook = lambda h: None
    sys.modules["antenv.axon_hooks"] = mod


def run(inputs, **kwargs):
    if kwargs.get("trace"):
        _ensure_ntff_hook()
    in_maps, metas = host_prep(inputs)
    nc = _get_nc()
    res = run_bass_kernel_spmd(nc, in_maps, core_ids=list(range(NCORE)),
                               **kwargs)
    out = np.zeros((N_RAYS, 3), np.float32)
    for c, (r0, r1, n_rays) in enumerate(metas):
        o = res.results[c]["outrgb"].reshape(NRT * 128, 3)
        out[r0:r1] = o[:n_rays]
    return out, res


def kernel(**inputs):
    return run(inputs)[0]


# revision 31
# speedup vs baseline: 1.6679x; 1.6679x over previous
"""Trainium2 Bass kernel for nn_DVGOMoE (moe_routing).

Strategy
--------
Data-parallel over rays: the 8192 rays (1048576 points, sorted by ray_id) are
split into 8 contiguous ray ranges with ~equal point counts; each NeuronCore
processes its range independently (rays never span cores). Gate + expert MLP
weights are replicated on every core.

Per core, two passes:

Pass 1 (point-major, tiled by 2048-point superchunks):
  * x^T is staged host-side as [6, pts] feature-major chunks at 4 partition
    groups (rows 32i..32i+5) so 4 row-tiled K=6 matmuls run concurrently on
    the PE array.
  * gate logits via stationary-x^T matmuls -> [128 pts, 8] point-major; top-2
    + renormalization computed with reduce_max / is_equal one-hots and
    sigma(z1-z2) = exp(-ln(1+exp(-(z1-z2)))).
  * per expert e (8 experts + density "expert" 8): h_e^T = relu(We1_e^T x + b)
    via row-tiled matmuls -> PSUM, evacuated (fused bias+relu) to SBUF split
    across DVE and ACT; second matmul with stationary h_e^T block and
    streamed [Walpha_e|Wrgb_e] producing point-major [128, 4] outputs.
  * all nonlinearities use only Exp/Ln (one ACT table set):
      sigmoid(z) = exp(-ln(1+exp(-z)))
      raw2alpha(y) = 1 - (1+exp(y+ACT_SHIFT))^(-1/2)    [INTERVAL = 0.5]
                   = 1 - exp(-0.5*ln(1+exp(y+ACT_SHIFT)))
  * gate-mixed quantities (sum_e g_e * q_e, sum_e g_e * rgb_e) and the
    density alpha0 are staged to DRAM per point (5 floats/point).

Pass 2 (ray-major): rays are padded to S=192 slots. For each tile of 128
rays, the 5 staged quantities are gathered with indirect DMA (row r =
staging[start_r : start_r+192], tail masked). Transmittance is an inclusive
per-ray cumsum of ln(1-alpha) via tensor_tensor_scan; weights, the keep mask
(w0 > 1e-4, folded multiplicatively), weighted rgb sums (free-dim reduce) and
the background composite all stay per-partition. Output is [128, 3] per tile.
"""

import numpy as np

import concourse.bass as bass
import concourse.bacc as bacc
import concourse.tile as tile
from concourse import mybir
from concourse.bass_utils import run_bass_kernel_spmd
from concourse.tile_rust import add_dep_helper

F32 = mybir.dt.float32
I32 = mybir.dt.int32
AF = mybir.ActivationFunctionType
OP = mybir.AluOpType
AX = mybir.AxisListType

# problem constants
P_TOT = 1048576
N_RAYS = 8192
E = 8
H = 128
ACT_SHIFT = -4.0
THRESH = 1e-4
NCORE = 8

# kernel layout constants
SC = 2048                 # points per superchunk
NSC = 65                  # superchunks per core
P_PAD = NSC * SC          # padded points per core (133120)
NRT = 9                   # ray tiles per core
R_PAD = NRT * 128         # padded rays per core (1152)
S = 192                   # slots per ray (max ray len 174 in this data)
STG = P_PAD + 256         # staging length (gather overrun margin)

_H_DT = mybir.dt.bfloat16  # FWL halves mm2 LDWEIGHTS (the PE bottleneck)


def build_nc(nsc=NSC, nrt=NRT, h_dt=_H_DT, num_devices=NCORE):
    nc = bacc.Bacc("TRN2", target_bir_lowering=False, debug=False,
                   num_devices=num_devices)

    # ---- DRAM I/O ----
    xt = nc.dram_tensor("xt", [nsc, 24, 512], F32, kind="ExternalInput")
    w1 = nc.dram_tensor("w1", [128, 9, 128], F32, kind="ExternalInput")
    wg = nc.dram_tensor("wg", [128, 8], F32, kind="ExternalInput")
    w2 = nc.dram_tensor("w2", [128, 36], F32, kind="ExternalInput")
    be1 = nc.dram_tensor("be1", [128, 9], F32, kind="ExternalInput")
    bgc = nc.dram_tensor("bgc", [128, 3], F32, kind="ExternalInput")
    starts = nc.dram_tensor("starts", [nrt, 128], I32, kind="ExternalInput")
    pmask = nc.dram_tensor("pmask", [nrt, 128, S], F32, kind="ExternalInput")
    outrgb = nc.dram_tensor("outrgb", [nrt, 128, 3], F32, kind="ExternalOutput")

    stg_len = nsc * SC + 256
    stg_names = ["sa0", "sq", "sr", "sg2", "sb2"]
    stg = [nc.dram_tensor(n, [stg_len], F32, kind="Internal")
           for n in stg_names]

    stg_sem_cm = nc.semaphore()
    stg_sem = stg_sem_cm.__enter__()

    with tile.TileContext(nc) as tc:
        consts = tc.alloc_tile_pool(name="consts", bufs=1)
        sb = tc.alloc_tile_pool(name="sb", bufs=3)
        xap = tc.alloc_tile_pool(name="xap", bufs=4)
        hpool = tc.alloc_tile_pool(name="hpool", bufs=3)
        ps = tc.alloc_tile_pool(name="ps", bufs=1, space="PSUM")
        hps = tc.alloc_tile_pool(name="hps", bufs=1, space="PSUM")
        ps2 = tc.alloc_tile_pool(name="ps2", bufs=2, space="PSUM")

        # ---- load constants ----
        w1c = consts.tile([128, 9, 128], F32)
        wgc = consts.tile([128, 8], F32)
        w2c = consts.tile([128, 36], h_dt)
        be1c = consts.tile([128, 9], F32)
        bgcc = consts.tile([128, 3], F32)
        onesS = consts.tile([128, S], F32)
        shiftc = consts.tile([128, 1], F32)
        nc.vector.memset(shiftc[:], ACT_SHIFT)
        nc.sync.dma_start(w1c[:], w1.ap())
        nc.sync.dma_start(wgc[:], wg.ap())
        if h_dt == F32:
            nc.sync.dma_start(w2c[:], w2.ap())
        else:
            nc.gpsimd.dma_start(w2c[:], w2.ap())  # SWDGE casts f32 -> bf16
        nc.sync.dma_start(be1c[:], be1.ap())
        nc.sync.dma_start(bgcc[:], bgc.ap())
        nc.vector.memset(onesS[:], 1.0)

        # zero the staging overrun tail (gathers read past the last point;
        # uninitialized DRAM could be non-finite and NaN*0 = NaN)
        stg_writes = [[] for _ in range(5)]  # explicit RAW deps for gathers
        n_stg_writes = [0]
        ztail = consts.tile([128, 2], F32)
        nc.vector.memset(ztail[:], 0.0)
        for qi in range(5):
            tail = stg[qi].ap()[bass.ds(nsc * SC, 256)].rearrange(
                "(b p) -> p b", p=128)
            w = nc.sync.dma_start(tail, ztail[:])
            stg_writes[qi].append(w.ins)
            n_stg_writes[0] += 1

        xt_r = xt.ap().rearrange("a r f -> (a r) f")

        # ================= PASS 1 =================
        def sc_body(s):
            # x^T superchunk: rows 32i..32i+5 = chunk i features
            xa = xap.tile([128, 512], F32, tag="xa", name="xa")
            for i in range(4):
                nc.sync.dma_start(
                    xa[32 * i:32 * i + 6, :],
                    xt_r[bass.ds(s * 24 + 6 * i, 6), :])

            # ---- gate: stationary-x^T blocks, out [128 pts, 8] ----
            zg_ps = ps2.tile([128, 16, 8], F32, tag="zg", name="zg_ps")
            for i in range(4):
                for b in range(4):
                    k = 4 * i + b
                    nc.tensor.matmul(
                        zg_ps[:, k, :],
                        lhsT=xa[32 * i:32 * i + 6, 128 * b:128 * (b + 1)],
                        rhs=wgc[32 * i:32 * i + 6, :],
                        start=True, stop=True, tile_position=(32 * i, 0))
            zg = sb.tile([128, 16, 8], F32, tag="zg_sb", name="zg")
            nc.vector.tensor_copy(zg[:], zg_ps[:])

            # top-2 one-hots + weights
            m1 = sb.tile([128, 16], F32, tag="m1", name="m1")
            nc.vector.tensor_reduce(out=m1[:], in_=zg[:], axis=AX.X, op=OP.max)
            m1b = bass.AP(m1.tensor, m1[:].offset,
                          [m1[:].ap[0], m1[:].ap[1], [0, 8]])
            eq1 = sb.tile([128, 16, 8], F32, tag="eq1", name="eq1")
            nc.vector.tensor_tensor(out=eq1[:], in0=zg[:], in1=m1b,
                                    op=OP.is_equal)
            zm = sb.tile([128, 16, 8], F32, tag="zm", name="zm")
            nc.vector.scalar_tensor_tensor(out=zm[:], in0=eq1[:], scalar=-1e9,
                                           in1=zg[:], op0=OP.mult, op1=OP.add)
            m2 = sb.tile([128, 16], F32, tag="m2", name="m2")
            nc.vector.tensor_reduce(out=m2[:], in_=zm[:], axis=AX.X, op=OP.max)
            m2b = bass.AP(m2.tensor, m2[:].offset,
                          [m2[:].ap[0], m2[:].ap[1], [0, 8]])
            eq2 = sb.tile([128, 16, 8], F32, tag="eq2", name="eq2")
            nc.vector.tensor_tensor(out=eq2[:], in0=zg[:], in1=m2b,
                                    op=OP.is_equal)
            d12 = sb.tile([128, 16], F32, tag="d12", name="d12")
            nc.vector.tensor_tensor(out=d12[:], in0=m1[:], in1=m2[:],
                                    op=OP.subtract)
            # g1 = sigmoid(d12) via exp/ln
            ge = sb.tile([128, 16], F32, tag="ge", name="ge")
            nc.scalar.activation(ge[:], d12[:], AF.Exp, bias=0.0, scale=-1.0)
            gl = sb.tile([128, 16], F32, tag="gl", name="gl")
            nc.scalar.activation(gl[:], ge[:], AF.Ln, bias=1.0, scale=1.0)
            g1 = sb.tile([128, 16], F32, tag="g1", name="g1")
            nc.scalar.activation(g1[:], gl[:], AF.Exp, bias=0.0, scale=-1.0)
            g1b = bass.AP(g1.tensor, g1[:].offset,
                          [g1[:].ap[0], g1[:].ap[1], [0, 8]])
            # g = eq2 + (eq1 - eq2) * g1
            dq = sb.tile([128, 16, 8], F32, tag="dq", name="dq")
            nc.vector.tensor_tensor(out=dq[:], in0=eq1[:], in1=eq2[:],
                                    op=OP.subtract)
            p1t = sb.tile([128, 16, 8], F32, tag="p1t", name="p1t")
            nc.vector.tensor_tensor(out=p1t[:], in0=dq[:], in1=g1b, op=OP.mult)
            gat = sb.tile([128, 16, 8], F32, tag="gat", name="gat")
            nc.vector.tensor_tensor(out=gat[:], in0=p1t[:], in1=eq2[:],
                                    op=OP.add)

            # ---- experts ----
            o_psA = ps.tile([128, 8, 36], F32, tag="oA", name="o_psA")
            o_psB = ps.tile([128, 8, 36], F32, tag="oB", name="o_psB")
            for e in range(9):
                # bf16 h: [128,512]bf16 = half bank, so 4 chunks fit in 2
                # banks and hbufs=2 double-buffers experts in 4 banks total
                hp = [hps.tile([128, 512], F32, tag=f"hp{i}", name=f"hp{i}")
                      for i in range(4)]
                for i in range(4):
                    nc.tensor.matmul(
                        hp[i][:], lhsT=w1c[32 * i:32 * i + 6, e, :],
                        rhs=xa[32 * i:32 * i + 6, :],
                        start=True, stop=True, tile_position=(32 * i, 0))
                hsb = hpool.tile([128, 2048], h_dt, tag="hsb", name="hsb")
                for i in range(4):
                    dst = hsb[:, 512 * i:512 * (i + 1)]
                    if i % 2 == 0:
                        nc.vector.tensor_scalar(
                            out=dst, in0=hp[i][:], scalar1=be1c[:, e:e + 1],
                            scalar2=0.0, op0=OP.add, op1=OP.max)
                    else:
                        nc.scalar.activation(dst, hp[i][:], AF.Relu,
                                             bias=be1c[:, e:e + 1], scale=1.0)
                for k in range(16):
                    o_ps = o_psA if k < 8 else o_psB
                    nc.tensor.matmul(
                        o_ps[:, k % 8, 4 * e:4 * e + 4],
                        lhsT=hsb[:, 128 * k:128 * (k + 1)],
                        rhs=w2c[:, 4 * e:4 * e + 4],
                        start=True, stop=True)

            # ---- nonlinearities (Exp/Ln only) ----
            vA = o_psA[:].rearrange("p b (s c) -> p b s c", c=4)
            vB = o_psB[:].rearrange("p b (s c) -> p b s c", c=4)
            ua = sb.tile([128, 16, 9], F32, tag="ua", name="ua")
            nc.scalar.activation(ua[:, 0:8, :], vA[:, :, :, 0], AF.Exp,
                                 bias=shiftc[:, :1], scale=1.0)
            nc.scalar.activation(ua[:, 8:16, :], vB[:, :, :, 0], AF.Exp,
                                 bias=shiftc[:, :1], scale=1.0)
            ur = sb.tile([128, 16, 9, 3], F32, tag="ur", name="ur")
            nc.scalar.activation(ur[:, 0:8, :, :], vA[:, :, :, 1:4], AF.Exp,
                                 bias=0.0, scale=-1.0)
            nc.scalar.activation(ur[:, 8:16, :, :], vB[:, :, :, 1:4], AF.Exp,
                                 bias=0.0, scale=-1.0)
            va = sb.tile([128, 16, 9], F32, tag="va", name="va")
            nc.scalar.activation(va[:], ua[:], AF.Ln, bias=1.0, scale=1.0)
            qa = sb.tile([128, 16, 9], F32, tag="qa", name="qa")
            nc.scalar.activation(qa[:], va[:], AF.Exp, bias=0.0, scale=-0.5)
            vr = sb.tile([128, 16, 9, 3], F32, tag="vr", name="vr")
            nc.scalar.activation(vr[:], ur[:], AF.Ln, bias=1.0, scale=1.0)
            sr = sb.tile([128, 16, 9, 3], F32, tag="sr", name="sr")
            nc.scalar.activation(sr[:], vr[:], AF.Exp, bias=0.0, scale=-1.0)

            # ---- gate mixing ----
            gq = sb.tile([128, 16, 8], F32, tag="gq", name="gq")
            nc.vector.tensor_tensor(out=gq[:], in0=qa[:, :, 0:8], in1=gat[:],
                                    op=OP.mult)
            umq = sb.tile([128, 16], F32, tag="umq", name="umq")
            nc.vector.tensor_reduce(out=umq[:], in_=gq[:], axis=AX.X, op=OP.add)
            ga3 = bass.AP(gat.tensor, gat[:].offset, gat[:].ap + [[0, 3]])
            pr = sb.tile([128, 16, 8, 3], F32, tag="pr", name="pr")
            nc.vector.tensor_tensor(out=pr[:], in0=sr[:, :, 0:8, :], in1=ga3,
                                    op=OP.mult)
            umr = sb.tile([128, 16, 3], F32, tag="umr", name="umr")
            nc.vector.tensor_reduce(out=umr[:],
                                    in_=pr[:].rearrange("p k e c -> p k c e"),
                                    axis=AX.X, op=OP.add)
            # density alpha0, thresholded
            a0 = sb.tile([128, 16], F32, tag="a0", name="a0")
            nc.vector.tensor_scalar(out=a0[:], in0=qa[:, :, 8], scalar1=-1.0,
                                    scalar2=1.0, op0=OP.mult, op1=OP.add)
            th = sb.tile([128, 16], F32, tag="th", name="th")
            nc.vector.tensor_scalar(out=th[:], in0=a0[:], scalar1=THRESH,
                                    scalar2=None, op0=OP.is_gt)
            a0f = sb.tile([128, 16], F32, tag="a0f", name="a0f")
            nc.vector.tensor_tensor(out=a0f[:], in0=a0[:], in1=th[:],
                                    op=OP.mult)

            # ---- stage to DRAM ----
            def stage(qi, src_ap):
                dst = stg[qi].ap()[bass.ds(s * SC, SC)].rearrange(
                    "(b p) -> p b", p=128)
                w = nc.sync.dma_start(dst, src_ap)
                stg_writes[qi].append(w.ins)
                n_stg_writes[0] += 1

            stage(0, a0f[:])
            stage(1, umq[:])
            stage(2, umr[:, :, 0])
            stage(3, umr[:, :, 1])
            stage(4, umr[:, :, 2])

        for s in range(nsc):
            sc_body(s)

        # ================= PASS 2 =================
        # hard fence: every pass-2 instruction (incl. the indirect gathers)
        # must observe all pass-1 staging writes
        tc.strict_bb_all_engine_barrier()
        p2 = tc.alloc_tile_pool(name="p2", bufs=2)
        for t in range(nrt):
            st = p2.tile([128, 1], I32, tag="st", name="st")
            nc.sync.dma_start(st[:], starts.ap()[t, :, None])
            msk = p2.tile([128, S], F32, tag="msk", name="msk")
            nc.sync.dma_start(msk[:], pmask.ap()[t])
            gth = []
            for qi in range(5):
                gt = p2.tile([128, S], F32, tag=f"g{qi}", name=f"gt{qi}")
                # [stg_len, 1] view: indirect coef=1, S elems/idx from out
                src = bass.AP(stg[qi], 0, [[1, stg_len], [1, 1]])
                gi = nc.gpsimd.indirect_dma_start(
                    out=gt[:], out_offset=None, in_=src,
                    in_offset=bass.IndirectOffsetOnAxis(ap=st[:, :1], axis=0))
                # Tile only tracks the most recent DRAM writer for the
                # indirect source; add the full RAW edge set explicitly.
                for w in stg_writes[qi]:
                    add_dep_helper(gi.ins, w, reason="staging RAW")
                gth.append(gt)
            pa0, pq, prr, pgg, pbb = gth

            # density pass: w0 and keep
            a0m = p2.tile([128, S], F32, tag="a0m", name="a0m")
            nc.vector.tensor_tensor(out=a0m[:], in0=pa0[:], in1=msk[:],
                                    op=OP.mult)
            la0 = p2.tile([128, S], F32, tag="la0", name="la0")
            nc.scalar.activation(la0[:], a0m[:], AF.Ln, bias=1.0, scale=-1.0)
            lc0 = p2.tile([128, S], F32, tag="lc0", name="lc0")
            nc.vector.tensor_tensor_scan(lc0[:], data0=onesS[:], data1=la0[:],
                                         initial=0.0, op0=OP.mult, op1=OP.add)
            ex0 = p2.tile([128, S], F32, tag="ex0", name="ex0")
            nc.vector.tensor_tensor(out=ex0[:], in0=lc0[:], in1=la0[:],
                                    op=OP.subtract)
            t0 = p2.tile([128, S], F32, tag="t0", name="t0")
            nc.scalar.activation(t0[:], ex0[:], AF.Exp, bias=0.0, scale=1.0)
            w0 = p2.tile([128, S], F32, tag="w0", name="w0")
            nc.vector.tensor_tensor(out=w0[:], in0=a0m[:], in1=t0[:],
                                    op=OP.mult)
            keep = p2.tile([128, S], F32, tag="keep", name="keep")
            nc.vector.tensor_scalar(out=keep[:], in0=w0[:], scalar1=THRESH,
                                    scalar2=None, op0=OP.is_gt)

            # alpha_mix = keep * (1 - umq), clipped
            am = p2.tile([128, S], F32, tag="am", name="am")
            nc.vector.tensor_scalar(out=am[:], in0=pq[:], scalar1=-1.0,
                                    scalar2=1.0, op0=OP.mult, op1=OP.add)
            amk = p2.tile([128, S], F32, tag="amk", name="amk")
            nc.vector.tensor_tensor(out=amk[:], in0=am[:], in1=keep[:],
                                    op=OP.mult)
            nc.vector.tensor_scalar(out=amk[:], in0=amk[:],
                                    scalar1=1.0 - 1e-6, scalar2=None,
                                    op0=OP.min)

            la2 = p2.tile([128, S], F32, tag="la2", name="la2")
            nc.scalar.activation(la2[:], amk[:], AF.Ln, bias=1.0, scale=-1.0)
            lc2 = p2.tile([128, S], F32, tag="lc2", name="lc2")
            nc.vector.tensor_tensor_scan(lc2[:], data0=onesS[:], data1=la2[:],
                                         initial=0.0, op0=OP.mult, op1=OP.add)
            ex2 = p2.tile([128, S], F32, tag="ex2", name="ex2")
            nc.vector.tensor_tensor(out=ex2[:], in0=lc2[:], in1=la2[:],
                                    op=OP.subtract)
            tt = p2.tile([128, S], F32, tag="tt", name="tt")
            nc.scalar.activation(tt[:], ex2[:], AF.Exp, bias=0.0, scale=1.0)
            ww = p2.tile([128, S], F32, tag="ww", name="ww")
            nc.vector.tensor_tensor(out=ww[:], in0=amk[:], in1=tt[:],
                                    op=OP.mult)

            # weighted rgb sums
            sums = p2.tile([128, 3], F32, tag="sums", name="sums")
            for ci, pc in enumerate((prr, pgg, pbb)):
                wr = p2.tile([128, S], F32, tag="wr", name="wr")
                nc.vector.tensor_tensor(out=wr[:], in0=pc[:], in1=ww[:],
                                        op=OP.mult)
                nc.vector.tensor_reduce(out=sums[:, ci:ci + 1], in_=wr[:],
                                        axis=AX.X, op=OP.add)
            ainv = p2.tile([128, 1], F32, tag="ainv", name="ainv")
            nc.scalar.activation(ainv[:], lc2[:, S - 1:S], AF.Exp,
                                 bias=0.0, scale=1.0)
            outt = p2.tile([128, 3], F32, tag="outt", name="outt")
            nc.vector.scalar_tensor_tensor(out=outt[:], in0=bgcc[:],
                                           scalar=ainv[:, :1], in1=sums[:],
                                           op0=OP.mult, op1=OP.add)
            nc.sync.dma_start(outrgb.ap()[t], outt[:])

        p2.release(); ps2.release(); hps.release(); ps.release()
        hpool.release(); xap.release(); sb.release(); consts.release()

    nc.compile()
    return nc


# ---------------- host-side prep ----------------

def host_prep(inputs, nsc=NSC, nrt=NRT):
    """Build per-core in_maps + metadata from full inputs."""
    ray_pts = np.asarray(inputs["ray_pts"], np.float32)
    viewdirs = np.asarray(inputs["viewdirs"], np.float32)
    bg = np.asarray(inputs["bg"], np.float32)
    Wd1 = np.asarray(inputs["Wd1"], np.float32)
    bd1 = np.asarray(inputs["bd1"], np.float32)
    Wd2 = np.asarray(inputs["Wd2"], np.float32)
    Wg = np.asarray(inputs["Wg"], np.float32)
    We1 = np.asarray(inputs["We1"], np.float32)
    be1 = np.asarray(inputs["be1"], np.float32)
    Wrgb = np.asarray(inputs["Wrgb"], np.float32)
    Walpha = np.asarray(inputs["Walpha"], np.float32)
    ray_id = np.asarray(inputs["ray_id"], np.int32)

    P = ray_pts.shape[0]
    N = viewdirs.shape[0]
    counts = np.bincount(ray_id, minlength=N).astype(np.int64)
    ends_cum = np.cumsum(counts)
    starts_all = ends_cum - counts

    # ray-boundary sharding with ~equal point counts
    tgt = np.arange(1, NCORE) * (P / NCORE)
    cut_rays = np.searchsorted(ends_cum, tgt)  # ray index where core ends
    ray_bounds = np.concatenate([[0], cut_rays + 1, [N]])
    pt_bounds = np.concatenate([[0], ends_cum[ray_bounds[1:-1] - 1], [P]])

    # shared constant blocks
    w1 = np.zeros((128, 9, 128), np.float32)
    we_all = np.concatenate(
        [We1, np.concatenate([Wd1, np.zeros((3, H), np.float32)], 0)[None]], 0)
    for i in range(4):
        w1[32 * i:32 * i + 6] = we_all.transpose(1, 0, 2)  # [6, 9, 128]
    wg = np.zeros((128, 8), np.float32)
    for i in range(4):
        wg[32 * i:32 * i + 6] = Wg
    w2 = np.zeros((128, 36), np.float32)
    for e in range(E):
        w2[:, 4 * e] = Walpha[e, :, 0]
        w2[:, 4 * e + 1:4 * e + 4] = Wrgb[e]
    w2[:, 32] = Wd2[:, 0]
    be1s = np.zeros((128, 9), np.float32)
    be1s[:, :8] = be1.T
    be1s[:, 8] = bd1
    bgc = np.broadcast_to(bg[None, :], (128, 3)).copy()

    in_maps, metas = [], []
    for c in range(NCORE):
        r0, r1 = int(ray_bounds[c]), int(ray_bounds[c + 1])
        p0, p1 = int(pt_bounds[c]), int(pt_bounds[c + 1])
        n_pts, n_rays = p1 - p0, r1 - r0
        assert n_pts <= nsc * SC, (n_pts, nsc * SC)
        assert n_rays <= nrt * 128, (n_rays, nrt * 128)

        xt = np.zeros((nsc * SC, 6), np.float32)
        xt[:n_pts, 0:3] = ray_pts[p0:p1]
        xt[:n_pts, 3:6] = viewdirs[ray_id[p0:p1]]
        # [NSC, 24, 512]: row 6i+j = feature j of chunk i
        xt = xt.reshape(nsc, 4, 512, 6).transpose(0, 1, 3, 2).reshape(
            nsc, 24, 512)

        st = np.zeros(nrt * 128, np.int32)
        st[:n_rays] = (starts_all[r0:r1] - p0).astype(np.int32)
        pm = np.zeros((nrt * 128, S), np.float32)
        lens = counts[r0:r1]
        pm[:n_rays] = (np.arange(S)[None, :] < lens[:, None])

        in_maps.append({
            "xt": xt, "w1": w1, "wg": wg, "w2": w2, "be1": be1s, "bgc": bgc,
            "starts": st.reshape(nrt, 128),
            "pmask": pm.reshape(nrt, 128, S),
        })
        metas.append((r0, r1, n_rays))
    return in_maps, metas


_NC_CACHE = {}


def _get_nc():
    key = (NSC, NRT, _H_DT)
    if key not in _NC_CACHE:
        _NC_CACHE[key] = build_nc()
    return _NC_CACHE[key]


def _ensure_ntff_hook():
    """Provide antenv.axon_hooks (NTFF profiling shim) if the image lacks it."""
    try:
        from antenv import axon_hooks  # noqa: F401
        return
    except ImportError:
        pass
    import sys
    import types
    try:
        from trn_agent_boot.trn_boot import _ntff_profile_via_ctypes
        hook = _ntff_profile_via_ctypes("/opt/axon/libaxon_pjrt.so")
    except Exception:
        hook = None
    mod = types.ModuleType("antenv.axon_hooks")
    mod.get_axon_ntff_profile_hook = lambda: hook
    mod.set_axon_ntff_profile_hook = lambda h: None
    sys.modules["antenv.axon_hooks"] = mod


def run(inputs, **kwargs):
    if kwargs.get("trace"):
        _ensure_ntff_hook()
    in_maps, metas = host_prep(inputs)
    nc = _get_nc()
    res = run_bass_kernel_spmd(nc, in_maps, core_ids=list(range(NCORE)),
                               **kwargs)
    out = np.zeros((N_RAYS, 3), np.float32)
    for c, (r0, r1, n_rays) in enumerate(metas):
        o = res.results[c]["outrgb"].reshape(NRT * 128, 3)
        out[r0:r1] = o[:n_rays]
    return out, res


def kernel(**inputs):
    return run(inputs)[0]
